# revision 1
# baseline (speedup 1.0000x reference)
"""Distributed Trainium2 kernel for causal GQA attention with RoPE.

Model: B=2, S=2048, DM=2048, H=16 q-heads, HK=4 kv-heads, D=128.
Sharding over 8 NeuronCores: core c = (batch b=c//4, kv-head kh=c%4).
Each core computes its 4 q-heads / 1 kv-head of one batch end-to-end,
AllGathers attention outputs within its 4-core batch group, and applies
a column slice of Wo, producing out[b][:, kh*512:(kh+1)*512].

Schedule: projection chunk 0, then attention quarters 0..3 with the
NEXT chunk's projection groups emitted between attention heads — the
dependency-free projection matmuls absorb the exp-chain (scalar ACT)
latency that otherwise stalls the PE inside a quarter.  Wo matmuls run
as single-matmul fillers popped inside later quarters' PV loops.  One
AllGather per quarter; the last quarter splits into two head-pair
gathers so the tail hides under the Wo drain.
Queues: sync = x + og loads + out stores; scalar = wk/wq/wv loads,
rope PSUM copies, exp, ost copies; gpsimd = tables + wo load + rope
shifts + broadcasts + cin stores + collectives.
"""
import contextlib
import ctypes
import os
import sys
import types
from collections import deque

for _p in ("/opt/trn_rl_repo", "/root/.axon_site/_ro/trn_rl_repo"):
    if os.path.isdir(_p) and _p not in sys.path:
        sys.path.insert(0, _p)

import numpy as np
import ml_dtypes

import concourse.bass as bass
import concourse.mybir as mybir
import concourse.tile as tile
from concourse import bacc
from concourse.bass import ts, ds
from concourse.bass_utils import run_bass_kernel_spmd

BF16 = ml_dtypes.bfloat16
F32 = mybir.dt.float32
BF = mybir.dt.bfloat16

B, S, DM = 2, 2048, 2048
H, HK, D = 16, 4, 128
G = H // HK          # q heads per kv head (= heads per core)
THETA = 10000.0
N_CORES = 8
KT = DM // 128       # 16 K-tiles of the model dim
TOKB = S // 128      # 16 token blocks
TCH = S // 512       # 4 token chunks of 512
HD_CORE = G * D      # 512 output dims of q per core
NEG = -1.0e30

LAST_EXEC_TIME_NS = None
LAST_RESULTS = None


# ---------------------------------------------------------------- tracing
def _install_ntff_hook():
    """Make run_bass_kernel_spmd(trace=True) work in this container."""
    try:
        from antenv.axon_hooks import get_axon_ntff_profile_hook  # noqa: F401
        return True
    except ImportError:
        pass
    so_path = "/opt/axon/libaxon_pjrt.so"
    if not os.path.exists(so_path):
        return False
    lib = ctypes.CDLL(so_path)
    if not hasattr(lib, "axon_start_nrt_profile"):
        return False
    lib.axon_start_nrt_profile.argtypes = [ctypes.POINTER(ctypes.c_int64), ctypes.c_size_t]
    lib.axon_start_nrt_profile.restype = ctypes.c_int64
    lib.axon_stop_nrt_profile.argtypes = [ctypes.c_char_p]
    lib.axon_stop_nrt_profile.restype = ctypes.c_int64

    @contextlib.contextmanager
    def _hook(output_dir, device_ids):
        import jax
        jax.devices()
        if device_ids:
            ids = (ctypes.c_int64 * len(device_ids))(*device_ids)
            rc = lib.axon_start_nrt_profile(ids, len(device_ids))
        else:
            rc = lib.axon_start_nrt_profile(None, 0)
        if rc != 0:
            raise RuntimeError(f"axon_start_nrt_profile rc={rc}")
        try:
            yield
        finally:
            n = lib.axon_stop_nrt_profile(str(output_dir).encode())
            print(f"profile: {n} file(s) in {output_dir}", file=sys.stderr)

    mod = types.ModuleType("antenv.axon_hooks")
    holder = {"h": _hook}
    mod.set_axon_ntff_profile_hook = lambda h: holder.__setitem__("h", h)
    mod.get_axon_ntff_profile_hook = lambda: holder.get("h")
    sys.modules["antenv.axon_hooks"] = mod
    import antenv
    antenv.axon_hooks = mod
    import concourse.bass_utils as bu
    bu.upload_artifacts = lambda tmpdir: str(tmpdir)
    return True


# ---------------------------------------------------------------- graph
def build_nc():
    nc = bacc.Bacc("TRN2", target_bir_lowering=False, debug=False,
                   num_devices=N_CORES)

    xt = nc.dram_tensor("xt", [DM, S], BF, kind="ExternalInput").ap()
    wq = nc.dram_tensor("wq", [DM, HD_CORE], BF, kind="ExternalInput").ap()
    wk = nc.dram_tensor("wk", [DM, D], BF, kind="ExternalInput").ap()
    wv = nc.dram_tensor("wv", [DM, D], BF, kind="ExternalInput").ap()
    wo = nc.dram_tensor("wo", [DM, HD_CORE], BF, kind="ExternalInput").ap()
    cosq = nc.dram_tensor("cosq", [D, S], F32, kind="ExternalInput").ap()
    sinq = nc.dram_tensor("sinq", [D, S], F32, kind="ExternalInput").ap()
    cosk = nc.dram_tensor("cosk", [D, S], F32, kind="ExternalInput").ap()
    sink = nc.dram_tensor("sink", [D, S], F32, kind="ExternalInput").ap()
    out = nc.dram_tensor("out", [S, HD_CORE], F32, kind="ExternalOutput").ap()

    groups = [[0, 1, 2, 3], [4, 5, 6, 7]]

    with tile.TileContext(nc) as tc:
        with tc.tile_pool(name="const", bufs=1) as cpool, \
             tc.tile_pool(name="wts", bufs=1) as wpool, \
             tc.tile_pool(name="acts", bufs=1) as apool, \
             tc.tile_pool(name="xin", bufs=64) as xpool, \
             tc.tile_pool(name="work", bufs=2) as work, \
             tc.tile_pool(name="etwork", bufs=7) as etwork, \
             tc.tile_pool(name="ogp", bufs=22) as ogpool, \
             tc.tile_pool(name="stats", bufs=2) as stats, \
             tc.tile_pool(name="bcp", bufs=2) as bcpool, \
             tc.tile_pool(name="psmm", bufs=4, space="PSUM") as ps_mm, \
             tc.tile_pool(name="pspv", bufs=2, space="PSUM") as ps_pv, \
             tc.tile_pool(name="psden", bufs=1, space="PSUM") as ps_den, \
             tc.tile_pool(name="pswo", bufs=1, space="PSUM") as ps_wo, \
             tc.tile_pool(name="dram", bufs=1, space="DRAM") as dpool:

            # ---------------- constants
            # transposed causal mask: keep [k_row p, q_col j] iff j >= p
            cmaskT = cpool.tile([128, 128], F32, tag="cmaskT", name="cmaskT")
            nc.gpsimd.memset(cmaskT[:], 0.0)
            nc.gpsimd.affine_select(
                out=cmaskT[:], in_=cmaskT[:],
                compare_op=mybir.AluOpType.is_ge, fill=NEG,
                base=0, pattern=[[1, 128]], channel_multiplier=-1)
            ones_sb = cpool.tile([128, 1], BF, tag="ones", name="ones")
            nc.gpsimd.memset(ones_sb[:], 1.0)
            # warm the ACT exp table so the first real exp is fast
            warm_act = cpool.tile([128, 1], F32, tag="warm_act",
                                  name="warm_act")
            nc.gpsimd.memset(warm_act[:], 0.0)
            nc.scalar.activation(out=warm_act[:], in_=warm_act[:],
                                 func=mybir.ActivationFunctionType.Exp)
            # warm up the collective path early (gpsimd-issued input DMA so
            # it doesn't queue behind the sync-engine load stream)
            warm_in = dpool.tile([128, 4], F32, tag="warm_in", name="warm_in")
            warm_out = dpool.tile([4, 128, 4], F32, tag="warm_out",
                                  name="warm_out")
            nc.gpsimd.dma_start(out=warm_in[:], in_=cosq[0:128, 0:4])
            nc.gpsimd.collective_compute(
                "AllGather", mybir.AluOpType.bypass,
                replica_groups=groups,
                ins=[warm_in.opt()], outs=[warm_out.opt()])

            # ---------------- loads: sync = x; scalar = wk, wq, wv;
            # gpsimd = rope tables + wo (needed last)
            wq_sb = [wpool.tile([128, HD_CORE], BF, tag=f"wq{kt}",
                                name=f"wq{kt}") for kt in range(KT)]
            wk_sb = [wpool.tile([128, D], BF, tag=f"wk{kt}",
                                name=f"wk{kt}") for kt in range(KT)]
            wv_sb = [wpool.tile([128, D], BF, tag=f"wv{kt}",
                                name=f"wv{kt}") for kt in range(KT)]
            wo_sb = [wpool.tile([128, HD_CORE], BF, tag=f"wo{kt}",
                                name=f"wo{kt}") for kt in range(KT)]

            def load_xc(c):
                ts_ = [xpool.tile([128, 512], BF, tag="xc", name="xc")
                       for _ in range(KT)]
                for kt in range(KT):
                    nc.sync.dma_start(
                        out=ts_[kt][:],
                        in_=xt[ds(128 * kt, 128), ds(512 * c, 512)])
                return ts_

            for kt in range(KT):
                nc.scalar.dma_start(out=wk_sb[kt][:],
                                    in_=wk[ds(128 * kt, 128), :])
            xc_state = [load_xc(0)]
            for kt in range(KT):
                nc.scalar.dma_start(out=wq_sb[kt][:],
                                    in_=wq[ds(128 * kt, 128), :])
            # q and k share one table pair; the D^-0.5 q-scale is folded into
            # the exp's free scale parameter instead
            tbl = {}
            for name, src in (("cosk", cosk), ("sink", sink)):
                t = cpool.tile([D, S], F32, tag=name)
                nc.gpsimd.dma_start(out=t[:], in_=src[:])
                tbl[name] = t
            for kt in range(KT):
                nc.scalar.dma_start(out=wv_sb[kt][:],
                                    in_=wv[ds(128 * kt, 128), :])
            # remaining x chunks up front: the sync queue blocks on gather
            # waits later (og loads), which must not delay x deliveries
            xc_state += [load_xc(c) for c in range(1, TCH)]
            for kt in range(KT):
                nc.gpsimd.dma_start(out=wo_sb[kt][:],
                                    in_=wo[ds(128 * kt, 128), :])

            # ---------------- persistent activations
            qt_sb = [apool.tile([D, S], BF, tag=f"qt{h}", name=f"qt{h}")
                     for h in range(G)]
            kt_sb = apool.tile([D, S], BF, tag="kt", name="kt")
            vtok_sb = apool.tile([128, TOKB, D], BF, tag="vtok", name="vtok")

            # ---------------- projections + RoPE + direct token-major v
            def rope_store(raw_ps, c, dst_slice, cos_t, sin_t):
                # t2 = raw*cos straight from PSUM; the SBUF copy only feeds
                # the rotate-half shift (DMA cannot read PSUM)
                raw = work.tile([128, 512], F32, tag="qraw", name="qraw")
                nc.scalar.copy(raw[:], raw_ps)
                t2 = work.tile([128, 512], F32, tag="t2", name="t2")
                nc.vector.tensor_mul(t2[:], raw_ps,
                                     cos_t[:, ds(512 * c, 512)])
                sh = work.tile([128, 512], F32, tag="sh", name="sh")
                nc.gpsimd.dma_start(out=sh[0:64, :], in_=raw[64:128, :])
                nc.gpsimd.dma_start(out=sh[64:128, :], in_=raw[0:64, :])
                t1 = work.tile([128, 512], F32, tag="t1", name="t1")
                nc.vector.tensor_mul(t1[:], sh[:], sin_t[:, ds(512 * c, 512)])
                nc.vector.tensor_add(dst_slice, t1[:], t2[:])

            def proj_groups(c):
                """Chunk c's projection as 6 thunks (k, q0..q3, v) to emit
                between attention heads."""
                st = {}

                def get_xc():
                    if "xc" not in st:
                        st["xc"] = xc_state.pop(0)
                    return st["xc"]

                def g_k():
                    xc = get_xc()
                    ps = ps_mm.tile([128, 512], F32, tag="mm", name="mm")
                    for kt in range(KT):
                        nc.tensor.matmul(ps[:], wk_sb[kt][:], xc[kt][:],
                                         start=(kt == 0), stop=(kt == KT - 1))
                    rope_store(ps[:], c, kt_sb[:, ds(512 * c, 512)],
                               tbl["cosk"], tbl["sink"])

                def mk_q(h):
                    def g_q():
                        xc = get_xc()
                        ps = ps_mm.tile([128, 512], F32, tag="mm", name="mm")
                        for kt in range(KT):
                            nc.tensor.matmul(ps[:], wq_sb[kt][:, ts(h, 128)],
                                             xc[kt][:],
                                             start=(kt == 0),
                                             stop=(kt == KT - 1))
                        rope_store(ps[:], c, qt_sb[h][:, ds(512 * c, 512)],
                                   tbl["cosk"], tbl["sink"])
                    return g_q

                def g_v():
                    xc = get_xc()
                    ps = ps_mm.tile([128, 512], F32, tag="mm", name="mm")
                    for tb in range(4):
                        for kt in range(KT):
                            nc.tensor.matmul(ps[:, ts(tb, 128)],
                                             xc[kt][:, ts(tb, 128)],
                                             wv_sb[kt][:],
                                             start=(kt == 0),
                                             stop=(kt == KT - 1))
                    nc.vector.tensor_copy(out=vtok_sb[:, ds(4 * c, 4), :],
                                          in_=ps[:])

                return [g_k, mk_q(0), mk_q(1), mk_q(2), mk_q(3), g_v]

            # ---------------- collective buffers: one AllGather per quarter,
            # except quarter 3 (processed last) which splits into two
            # head-pair gathers so the tail gather hides under the Wo drain
            cin = [dpool.tile([D, G, 512], BF, tag=f"cin{t}", name=f"cin{t}")
                   for t in range(TCH)]
            cout = [dpool.tile([4, D, G, 512], BF, tag=f"cout{t}",
                               name=f"cout{t}") for t in range(TCH)]
            cin3 = [dpool.tile([D, 2, 512], BF, tag=f"cin3{p}",
                               name=f"cin3{p}") for p in range(2)]
            cout3 = [dpool.tile([4, D, 2, 512], BF, tag=f"cout3{p}",
                                name=f"cout3{p}") for p in range(2)]

            def cin_ap(qc, h):
                if qc == 3:
                    return cin3[h // 2][:, h % 2, :]
                return cin[qc][:, h, :]

            # ---------------- Wo: og loads + filler-granular matmuls
            og = {}

            def wo_loads(t):
                """Load all 16 gathered [D,512] tiles for quarter t into
                SBUF (sync queue; waits on the gather)."""
                ogs = [None] * KT
                order = sorted(range(KT), key=lambda kt: (kt % G >= 2, kt)) \
                    if t == 3 else range(KT)
                for kt in order:
                    r, h = divmod(kt, G)
                    o = ogpool.tile([128, 512], BF, tag="og", name="og")
                    if t == 3:
                        nc.sync.dma_start(out=o[:],
                                          in_=cout3[h // 2][r, :, h % 2, :])
                    else:
                        nc.sync.dma_start(out=o[:], in_=cout[t][r, :, h, :])
                    ogs[kt] = o
                og[t] = ogs

            pe_fill = deque()

            def queue_wo(t, pool=None, tag="wo"):
                """Enqueue quarter t's Wo work as single-matmul closures."""
                pool = pool or ps_wo
                idx_order = sorted(range(KT), key=lambda i: (i % G >= 2, i)) \
                    if t == 3 else list(range(KT))
                for tb in range(4):
                    state = {}

                    def mk(tb, pos, idx, pool, state):
                        def f():
                            if pos == 0:
                                state["pw"] = pool.tile([128, 512], F32,
                                                        tag=tag, name="wo")
                            nc.tensor.matmul(state["pw"][:],
                                             og[t][idx][:, ts(tb, 128)],
                                             wo_sb[idx][:],
                                             start=(pos == 0),
                                             stop=(pos == KT - 1))
                            if pos == KT - 1:
                                ost = work.tile([128, 512], F32, tag="ost",
                                                name="ost")
                                nc.vector.tensor_copy(out=ost[:],
                                                      in_=state["pw"][:])
                                nc.sync.dma_start(
                                    out=out[ds(512 * t + 128 * tb, 128), :],
                                    in_=ost[:])
                        return f

                    for pos, idx in enumerate(idx_order):
                        pe_fill.append(mk(tb, pos, idx, pool, state))

            def pop_fill(n):
                for _ in range(n):
                    if not pe_fill:
                        return
                    pe_fill.popleft()()

            # ---------------- attention
            def emit_st(h, qc, kb):
                """score block, transposed: [k 128, q<=512] -> exp -> et"""
                band = kb - 4 * qc
                et = etwork.tile([128, 512], BF, tag="et", name="et")
                sps = ps_mm.tile([128, 512], F32, tag="mm", name="mm")
                if band >= 0:
                    off = 128 * band
                    w = 512 - off
                    nc.tensor.matmul(sps[:, :w], kt_sb[:, ts(kb, 128)],
                                     qt_sb[h][:, ds(512 * qc + off, w)],
                                     start=True, stop=True)
                    nc.vector.tensor_add(sps[:, :128], sps[:, :128], cmaskT[:])
                    if off:
                        nc.vector.memset(et[:, :off], 0.0)
                    nc.scalar.activation(
                        out=et[:, ds(off, w)], in_=sps[:, :w],
                        func=mybir.ActivationFunctionType.Exp,
                        scale=float(D) ** -0.5)
                    return et, off
                nc.tensor.matmul(sps[:], kt_sb[:, ts(kb, 128)],
                                 qt_sb[h][:, ds(512 * qc, 512)],
                                 start=True, stop=True)
                nc.scalar.activation(
                    out=et[:], in_=sps[:],
                    func=mybir.ActivationFunctionType.Exp,
                    scale=float(D) ** -0.5)
                return et, 0

            def emit_attn(qc, enq, post):
                """enq: head -> [("loads", t) | ("wo", t)] run at head start.
                post: head -> [thunks] (projection groups) run at head end."""
                for h in range(G):
                    for act in enq.get(h, ()):
                        if act[0] == "loads":
                            wo_loads(act[1])
                        else:
                            queue_wo(act[1])
                    nkb = 4 * qc + 4
                    oT_ps = ps_pv.tile([128, 512], F32, tag="pv", name="pv")
                    den_ps = ps_den.tile([1, 512], F32, tag="den", name="den")
                    pend = [emit_st(h, qc, k) for k in range(min(3, nkb))]
                    ngrp = (nkb + 3) // 4
                    esum = None
                    for kb in range(nkb):
                        if kb + 3 < nkb:
                            pend.append(emit_st(h, qc, kb + 3))
                        et, off = pend.pop(0)
                        nc.tensor.matmul(oT_ps[:, ds(off, 512 - off)],
                                         vtok_sb[:, kb, :],
                                         et[:, ds(off, 512 - off)],
                                         start=(kb == 0), stop=(kb == nkb - 1))
                        pop_fill(2)
                        # denominator: sum groups of 4 et tiles on DVE, then
                        # one ones-matmul per group
                        gi, gj = divmod(kb, 4)
                        last_in_grp = (gj == 3 or kb == nkb - 1)
                        if gj == 0:
                            esum = et
                        else:
                            nsum = etwork.tile([128, 512], BF, tag="esum",
                                               name="esum", bufs=3)
                            nc.vector.tensor_add(nsum[:], esum[:], et[:])
                            esum = nsum
                        if last_in_grp:
                            nc.tensor.matmul(den_ps[:], ones_sb[:, 0:1],
                                             esum[:],
                                             start=(gi == 0),
                                             stop=(gi == ngrp - 1))
                    rec = stats.tile([1, 512], F32, tag="recq", name="recq")
                    nc.vector.reciprocal_approx_fast(out=rec[:],
                                                     in_=den_ps[:])
                    bcast = bcpool.tile([128, 512], F32, tag="bcast",
                                        name="bcast")
                    nc.gpsimd.partition_broadcast(bcast[:], rec[:])
                    otst = work.tile([128, 512], BF, tag="otst", name="otst")
                    nc.vector.tensor_mul(otst[:], oT_ps[:], bcast[:])
                    nc.gpsimd.dma_start(out=cin_ap(qc, h), in_=otst[:])
                    if qc == 3 and h % 2 == 1:
                        p = h // 2
                        nc.gpsimd.collective_compute(
                            "AllGather", mybir.AluOpType.bypass,
                            replica_groups=groups,
                            ins=[cin3[p].opt()], outs=[cout3[p].opt()])
                    pop_fill(4)
                    for g in post.get(h, ()):
                        g()
                if qc != 3:
                    nc.gpsimd.collective_compute(
                        "AllGather", mybir.AluOpType.bypass,
                        replica_groups=groups,
                        ins=[cin[qc].opt()], outs=[cout[qc].opt()])

            # ---------------- schedule
            for g in proj_groups(0):
                g()
            pg = {c: proj_groups(c) for c in (1, 2, 3)}
            emit_attn(0, {},
                      {0: pg[1][0:2], 1: pg[1][2:4], 2: pg[1][4:6]})
            emit_attn(1, {0: [("loads", 0)], 3: [("wo", 0)]},
                      {0: pg[2][0:2], 1: pg[2][2:4], 2: pg[2][4:6]})
            emit_attn(2, {0: [("loads", 1)], 3: [("wo", 1)]},
                      {0: pg[3][0:2], 1: pg[3][2:4], 2: pg[3][4:6]})
            emit_attn(3, {0: [("loads", 2)], 3: [("wo", 2)]}, {})
            pop_fill(len(pe_fill))
            wo_loads(3)
            queue_wo(3, pool=ps_pv, tag="pv")
            pop_fill(len(pe_fill))

    nc.finalize()
    return nc


_NC_CACHE = {}


def _get_nc():
    if "nc" not in _NC_CACHE:
        _NC_CACHE["nc"] = build_nc()
    return _NC_CACHE["nc"]


def _rope_tables():
    inv = 1.0 / (THETA ** (np.arange(0, D, 2, dtype=np.float64) / D))  # [64]
    pos = np.arange(S, dtype=np.float64)
    fr = pos[:, None] * inv[None, :]                 # [S, 64]
    emb = np.concatenate([fr, fr], axis=1)           # [S, D]
    cos = np.cos(emb).T.astype(np.float32)           # [D, S]
    sin = np.sin(emb).T.astype(np.float32)
    sgn = np.where(np.arange(D) < D // 2, -1.0, 1.0).astype(np.float32)[:, None]
    scale = np.float32(D ** -0.5)
    return (cos * scale, sin * sgn * scale,          # q tables (pre-scaled)
            cos.copy(), sin * sgn)                   # k tables


def kernel(x, Wq, Wk, Wv, Wo):
    global LAST_EXEC_TIME_NS, LAST_RESULTS
    nc = _get_nc()
    cq, sq, ck, sk = _rope_tables()
    in_maps = []
    for c in range(N_CORES):
        b, kh = c // 4, c % 4
        in_maps.append({
            "xt": np.ascontiguousarray(x[b].T).astype(BF16),
            "wq": np.ascontiguousarray(Wq[:, kh * HD_CORE:(kh + 1) * HD_CORE]).astype(BF16),
            "wk": np.ascontiguousarray(Wk[:, kh * D:(kh + 1) * D]).astype(BF16),
            "wv": np.ascontiguousarray(Wv[:, kh * D:(kh + 1) * D]).astype(BF16),
            "wo": np.ascontiguousarray(Wo[:, kh * HD_CORE:(kh + 1) * HD_CORE]).astype(BF16),
            "cosq": cq, "sinq": sq, "cosk": ck, "sink": sk,
        })
    trace = os.environ.get("KERNEL_TRACE", "0") == "1" and _install_ntff_hook()
    if os.environ.get("KERNEL_WARMUP", "1") == "1":
        # Untraced warm-up execution: first-launch NEFF load/JIT skews the 8
        # cores by 10-100us, which lands in core 0's collective waits.  A
        # warm-up run aligns the cores so the measured run reflects the
        # kernel, not launch jitter.
        run_bass_kernel_spmd(nc, in_maps, core_ids=list(range(N_CORES)),
                             trace=False)
    res = run_bass_kernel_spmd(nc, in_maps, core_ids=list(range(N_CORES)),
                               trace=trace)
    LAST_EXEC_TIME_NS = res.exec_time_ns
    LAST_RESULTS = res
    out = np.empty((B, S, DM), dtype=np.float32)
    for c in range(N_CORES):
        b, kh = c // 4, c % 4
        out[b, :, kh * HD_CORE:(kh + 1) * HD_CORE] = res.results[c]["out"]
    return out



# revision 4
# speedup vs baseline: 1.0102x; 1.0102x over previous
"""Distributed Trainium2 kernel for causal GQA attention with RoPE.

Model: B=2, S=2048, DM=2048, H=16 q-heads, HK=4 kv-heads, D=128.
Sharding over 8 NeuronCores: core c = (batch b=c//4, kv-head kh=c%4).
Each core computes its 4 q-heads / 1 kv-head of one batch end-to-end,
AllGathers attention outputs within its 4-core batch group, and applies
a column slice of Wo, producing out[b][:, kh*512:(kh+1)*512].

v2 schedule vs baseline (389.8us):
- startup: PE warm-up matmuls at t=0 (HAM), rope tables loaded per-chunk
  with chunk 0 first, wq loaded per-head so head 0 can project early,
  chunk-0 q1..q3 projections interleaved into quarter-0 attention.
- rope: no ACT copy; u = raw*sin_pre on DVE (bf16), DMA partition-shift
  of u, then dst = sh + raw*cos.  sin table pre-shifted on host.
- causal mask folded into the score matmul (lhsT = NEG upper-triangle,
  rhs = identity, accumulated before the k.T@q matmul) - no DVE mask add.
- softmax denominator: pairwise-tree DVE adds of the exp tiles, then one
  gpsimd partition_all_reduce per (head, quarter); no ones-matmul, no
  PSUM den bank, no partition_broadcast.
- PSUM: scores 3 bufs (lookahead 2), projections 2, PV 2, Wo 1.
- tail: quarter 3 runs one AllGather per head right after that head's
  output; Wo accumulates in 4 per-token-block PSUM tiles across four
  "waves" (one per gathered head) so only the last head's 16 matmuls
  trail the final gather.
"""
import contextlib
import ctypes
import os
import sys
import types
from collections import deque

for _p in ("/opt/trn_rl_repo", "/root/.axon_site/_ro/trn_rl_repo"):
    if os.path.isdir(_p) and _p not in sys.path:
        sys.path.insert(0, _p)

import numpy as np
import ml_dtypes

import concourse.bass as bass
import concourse.mybir as mybir
import concourse.tile as tile
from concourse import bacc, bass_isa
from concourse.bass import ts, ds
from concourse.bass_utils import run_bass_kernel_spmd

BF16 = ml_dtypes.bfloat16
F32 = mybir.dt.float32
BF = mybir.dt.bfloat16

B, S, DM = 2, 2048, 2048
H, HK, D = 16, 4, 128
G = H // HK          # q heads per kv head (= heads per core)
THETA = 10000.0
N_CORES = 8
KT = DM // 128       # 16 K-tiles of the model dim
TOKB = S // 128      # 16 token blocks
TCH = S // 512       # 4 token chunks of 512
HD_CORE = G * D      # 512 output dims of q per core
NEG = -1.0e30
LOOKAHEAD = 2        # score tiles in flight ahead of PV

LAST_EXEC_TIME_NS = None
LAST_RESULTS = None


# ---------------------------------------------------------------- tracing
def _install_ntff_hook():
    """Make run_bass_kernel_spmd(trace=True) work in this container."""
    try:
        from antenv.axon_hooks import get_axon_ntff_profile_hook  # noqa: F401
        return True
    except ImportError:
        pass
    so_path = "/opt/axon/libaxon_pjrt.so"
    if not os.path.exists(so_path):
        return False
    lib = ctypes.CDLL(so_path)
    if not hasattr(lib, "axon_start_nrt_profile"):
        return False
    lib.axon_start_nrt_profile.argtypes = [ctypes.POINTER(ctypes.c_int64), ctypes.c_size_t]
    lib.axon_start_nrt_profile.restype = ctypes.c_int64
    lib.axon_stop_nrt_profile.argtypes = [ctypes.c_char_p]
    lib.axon_stop_nrt_profile.restype = ctypes.c_int64

    @contextlib.contextmanager
    def _hook(output_dir, device_ids):
        import jax
        jax.devices()
        if device_ids:
            ids = (ctypes.c_int64 * len(device_ids))(*device_ids)
            rc = lib.axon_start_nrt_profile(ids, len(device_ids))
        else:
            rc = lib.axon_start_nrt_profile(None, 0)
        if rc != 0:
            raise RuntimeError(f"axon_start_nrt_profile rc={rc}")
        try:
            yield
        finally:
            n = lib.axon_stop_nrt_profile(str(output_dir).encode())
            print(f"profile: {n} file(s) in {output_dir}", file=sys.stderr)

    mod = types.ModuleType("antenv.axon_hooks")
    holder = {"h": _hook}
    mod.set_axon_ntff_profile_hook = lambda h: holder.__setitem__("h", h)
    mod.get_axon_ntff_profile_hook = lambda: holder.get("h")
    sys.modules["antenv.axon_hooks"] = mod
    import antenv
    antenv.axon_hooks = mod
    import concourse.bass_utils as bu
    bu.upload_artifacts = lambda tmpdir: str(tmpdir)
    return True


# ---------------------------------------------------------------- graph
def build_nc():
    nc = bacc.Bacc("TRN2", target_bir_lowering=False, debug=False,
                   num_devices=N_CORES)

    xt = nc.dram_tensor("xt", [DM, S], BF, kind="ExternalInput").ap()
    wq = nc.dram_tensor("wq", [DM, HD_CORE], BF, kind="ExternalInput").ap()
    wk = nc.dram_tensor("wk", [DM, D], BF, kind="ExternalInput").ap()
    wv = nc.dram_tensor("wv", [DM, D], BF, kind="ExternalInput").ap()
    wo = nc.dram_tensor("wo", [DM, HD_CORE], BF, kind="ExternalInput").ap()
    cosk = nc.dram_tensor("cosk", [D, S], F32, kind="ExternalInput").ap()
    sink = nc.dram_tensor("sink", [D, S], F32, kind="ExternalInput").ap()
    cmut = nc.dram_tensor("cmut", [128, 128], BF, kind="ExternalInput").ap()
    iden = nc.dram_tensor("iden", [128, 128], BF, kind="ExternalInput").ap()
    out = nc.dram_tensor("out", [S, HD_CORE], F32, kind="ExternalOutput").ap()

    groups = [[0, 1, 2, 3], [4, 5, 6, 7]]

    with tile.TileContext(nc) as tc:
        with tc.tile_pool(name="const", bufs=1) as cpool, \
             tc.tile_pool(name="wts", bufs=1) as wpool, \
             tc.tile_pool(name="acts", bufs=1) as apool, \
             tc.tile_pool(name="xin", bufs=64) as xpool, \
             tc.tile_pool(name="work", bufs=2) as work, \
             tc.tile_pool(name="etwork", bufs=7) as etwork, \
             tc.tile_pool(name="ogp", bufs=20) as ogpool, \
             tc.tile_pool(name="stats", bufs=2) as stats, \
             tc.tile_pool(name="pssc", bufs=3, space="PSUM") as ps_sc, \
             tc.tile_pool(name="pspj", bufs=2, space="PSUM") as ps_pj, \
             tc.tile_pool(name="pspv", bufs=2, space="PSUM") as ps_pv, \
             tc.tile_pool(name="pswo", bufs=1, space="PSUM") as ps_wo, \
             tc.tile_pool(name="dram", bufs=1, space="DRAM") as dpool:

            # ---------------- constants (host-built, tiny: land first)
            cmut_sb = cpool.tile([128, 128], BF, tag="cmut", name="cmut")
            nc.scalar.dma_start(out=cmut_sb[:], in_=cmut[:])
            iden_sb = cpool.tile([128, 128], BF, tag="iden", name="iden")
            nc.scalar.dma_start(out=iden_sb[:], in_=iden[:])

            # warm the ACT exp table so the first real exp is fast
            warm_act = cpool.tile([128, 1], F32, tag="warm_act",
                                  name="warm_act")
            nc.gpsimd.memset(warm_act[:], 0.0)
            nc.scalar.activation(out=warm_act[:], in_=warm_act[:],
                                 func=mybir.ActivationFunctionType.Exp)

            # PE warm-up: ~3.5us of dependency-free matmuls at t=0 so the
            # HAM clock-gate opens to 8/8 before the real projections land
            warm_rhs = cpool.tile([128, 512], BF, tag="warm_rhs",
                                  name="warm_rhs")
            nc.gpsimd.memset(warm_rhs[:], 0.0)
            warm_ps = ps_wo.tile([128, 512], F32, tag="wo", name="warm_ps")
            for _ in range(8):
                nc.tensor.matmul(warm_ps[:], warm_rhs[:, 0:128],
                                 warm_rhs[:], start=True, stop=True)

            # ---------------- rope tables: per-chunk tiles, chunk 0 first
            cos_t = [None] * TCH
            sin_t = [None] * TCH

            def load_tbl(c):
                ct = cpool.tile([D, 512], F32, tag=f"cos{c}", name=f"cos{c}")
                nc.gpsimd.dma_start(out=ct[:], in_=cosk[:, ds(512 * c, 512)])
                st = cpool.tile([D, 512], F32, tag=f"sin{c}", name=f"sin{c}")
                nc.gpsimd.dma_start(out=st[:], in_=sink[:, ds(512 * c, 512)])
                cos_t[c], sin_t[c] = ct, st

            load_tbl(0)
            # warm up the collective path early
            warm_in = dpool.tile([128, 4], F32, tag="warm_in", name="warm_in")
            warm_out = dpool.tile([4, 128, 4], F32, tag="warm_out",
                                  name="warm_out")
            nc.gpsimd.dma_start(out=warm_in[:], in_=cosk[0:128, 0:4])
            nc.gpsimd.collective_compute(
                "AllGather", mybir.AluOpType.bypass,
                replica_groups=groups,
                ins=[warm_in.opt()], outs=[warm_out.opt()])
            load_tbl(1)
            load_tbl(2)
            load_tbl(3)

            # ---------------- weights: scalar queue, critical first
            # wk -> wq head 0 -> wv -> wq heads 1-3.  wo loads are emitted
            # later (during quarter 0) so they don't steal startup HBM BW.
            wk_sb = [wpool.tile([128, D], BF, tag=f"wk{kt}",
                                name=f"wk{kt}") for kt in range(KT)]
            wq_sb = [[wpool.tile([128, 128], BF, tag=f"wq{h}_{kt}",
                                 name=f"wq{h}_{kt}") for kt in range(KT)]
                     for h in range(G)]
            wv_sb = [wpool.tile([128, D], BF, tag=f"wv{kt}",
                                name=f"wv{kt}") for kt in range(KT)]
            wo_sb = [wpool.tile([128, HD_CORE], BF, tag=f"wo{kt}",
                                name=f"wo{kt}") for kt in range(KT)]

            for kt in range(KT):
                nc.scalar.dma_start(out=wk_sb[kt][:],
                                    in_=wk[ds(128 * kt, 128), :])
            for kt in range(KT):
                nc.scalar.dma_start(out=wq_sb[0][kt][:],
                                    in_=wq[ds(128 * kt, 128), ts(0, 128)])
            for kt in range(KT):
                nc.scalar.dma_start(out=wv_sb[kt][:],
                                    in_=wv[ds(128 * kt, 128), :])
            for h in range(1, G):
                for kt in range(KT):
                    nc.scalar.dma_start(out=wq_sb[h][kt][:],
                                        in_=wq[ds(128 * kt, 128), ts(h, 128)])

            def load_xc(c):
                ts_ = [xpool.tile([128, 512], BF, tag="xc", name="xc")
                       for _ in range(KT)]
                for kt in range(KT):
                    nc.sync.dma_start(
                        out=ts_[kt][:],
                        in_=xt[ds(128 * kt, 128), ds(512 * c, 512)])
                return ts_

            xc_state = [load_xc(c) for c in range(TCH)]

            def load_wo(kt):
                nc.gpsimd.dma_start(out=wo_sb[kt][:],
                                    in_=wo[ds(128 * kt, 128), :])

            # ---------------- persistent activations
            qt_sb = [apool.tile([D, S], BF, tag=f"qt{h}", name=f"qt{h}")
                     for h in range(G)]
            kt_sb = apool.tile([D, S], BF, tag="kt", name="kt")
            vtok_sb = apool.tile([128, TOKB, D], BF, tag="vtok", name="vtok")

            # ---------------- projections + RoPE + direct token-major v
            def rope_store(raw_ps, c, dst_slice):
                # u = raw * sin_pre (sin table pre-shifted by 64 partitions
                # on host); shift u by 64 partitions via DMA; add raw*cos.
                u = work.tile([128, 512], BF, tag="u", name="u")
                nc.vector.tensor_mul(u[:], raw_ps, sin_t[c][:])
                t2 = work.tile([128, 512], BF, tag="t2", name="t2")
                nc.vector.tensor_mul(t2[:], raw_ps, cos_t[c][:])
                sh = work.tile([128, 512], BF, tag="sh", name="sh")
                nc.gpsimd.dma_start(out=sh[0:64, :], in_=u[64:128, :])
                nc.gpsimd.dma_start(out=sh[64:128, :], in_=u[0:64, :])
                nc.vector.tensor_add(dst_slice, sh[:], t2[:])

            def proj_groups(c):
                """Chunk c's projection as 6 thunks (k, q0..q3, v)."""
                st = {}

                def get_xc():
                    if "xc" not in st:
                        st["xc"] = xc_state[c]
                    return st["xc"]

                def g_k():
                    xc = get_xc()
                    ps = ps_pj.tile([128, 512], F32, tag="pj", name="pj")
                    for kt in range(KT):
                        nc.tensor.matmul(ps[:], wk_sb[kt][:], xc[kt][:],
                                         start=(kt == 0), stop=(kt == KT - 1))
                    rope_store(ps[:], c, kt_sb[:, ds(512 * c, 512)])

                def mk_q(h):
                    def g_q():
                        xc = get_xc()
                        ps = ps_pj.tile([128, 512], F32, tag="pj", name="pj")
                        for kt in range(KT):
                            nc.tensor.matmul(ps[:], wq_sb[h][kt][:],
                                             xc[kt][:],
                                             start=(kt == 0),
                                             stop=(kt == KT - 1))
                        rope_store(ps[:], c, qt_sb[h][:, ds(512 * c, 512)])
                    return g_q

                def g_v():
                    xc = get_xc()
                    ps = ps_pj.tile([128, 512], F32, tag="pj", name="pj")
                    for tb in range(4):
                        for kt in range(KT):
                            nc.tensor.matmul(ps[:, ts(tb, 128)],
                                             xc[kt][:, ts(tb, 128)],
                                             wv_sb[kt][:],
                                             start=(kt == 0),
                                             stop=(kt == KT - 1))
                    nc.vector.tensor_copy(out=vtok_sb[:, ds(4 * c, 4), :],
                                          in_=ps[:])

                return {"k": g_k, "q0": mk_q(0), "q1": mk_q(1),
                        "q2": mk_q(2), "q3": mk_q(3), "v": g_v}

            # ---------------- collective buffers
            # quarters 0-2: one gather of all 4 heads; quarter 3: one
            # gather per head, issued as soon as that head's output is done
            cin = [dpool.tile([D, G, 512], BF, tag=f"cin{t}", name=f"cin{t}")
                   for t in range(3)]
            cout = [dpool.tile([4, D, G, 512], BF, tag=f"cout{t}",
                               name=f"cout{t}") for t in range(3)]
            cin3 = [dpool.tile([D, 512], BF, tag=f"cin3{h}", name=f"cin3{h}")
                    for h in range(G)]
            cout3 = [dpool.tile([4, D, 512], BF, tag=f"cout3{h}",
                                name=f"cout3{h}") for h in range(G)]

            # ---------------- Wo: og loads + filler-granular matmuls
            og = {}

            def wo_loads(t):
                ogs = []
                for kt in range(KT):
                    r, h = divmod(kt, G)
                    o = ogpool.tile([128, 512], BF, tag="og", name="og")
                    nc.sync.dma_start(out=o[:], in_=cout[t][r, :, h, :])
                    ogs.append(o)
                og[t] = ogs

            pe_fill = deque()

            def queue_wo(t):
                """Enqueue quarter t's Wo work as single-matmul closures."""
                for tb in range(4):
                    state = {}

                    def mk(tb, pos, state):
                        def f():
                            if pos == 0:
                                state["pw"] = ps_wo.tile([128, 512], F32,
                                                         tag="wo", name="wo")
                            nc.tensor.matmul(state["pw"][:],
                                             og[t][pos][:, ts(tb, 128)],
                                             wo_sb[pos][:],
                                             start=(pos == 0),
                                             stop=(pos == KT - 1))
                            if pos == KT - 1:
                                ost = work.tile([128, 512], F32, tag="ost",
                                                name="ost", bufs=3)
                                nc.vector.tensor_copy(out=ost[:],
                                                      in_=state["pw"][:])
                                nc.sync.dma_start(
                                    out=out[ds(512 * t + 128 * tb, 128), :],
                                    in_=ost[:])
                        return f

                    for pos in range(KT):
                        pe_fill.append(mk(tb, pos, state))

            def pop_fill(n):
                for _ in range(n):
                    if not pe_fill:
                        return
                    pe_fill.popleft()()

            # ---------------- attention
            def emit_st(h, qc, kb):
                """score block, transposed: [k 128, q<=512] -> exp -> et.
                Causal mask accumulated via matmul (cmut.T @ iden)."""
                band = kb - 4 * qc
                et = etwork.tile([128, 512], BF, tag="et", name="et")
                sps = ps_sc.tile([128, 512], F32, tag="mm", name="mm")
                if band >= 0:
                    off = 128 * band
                    w = 512 - off
                    nc.tensor.matmul(sps[:, 0:128], cmut_sb[:], iden_sb[:],
                                     start=True, stop=False)
                    nc.tensor.matmul(sps[:, :w], kt_sb[:, ts(kb, 128)],
                                     qt_sb[h][:, ds(512 * qc + off, w)],
                                     start=False, stop=True)
                    if off:
                        nc.vector.memset(et[:, :off], 0.0)
                    nc.scalar.activation(
                        out=et[:, ds(off, w)], in_=sps[:, :w],
                        func=mybir.ActivationFunctionType.Exp,
                        scale=float(D) ** -0.5)
                    return et, off
                nc.tensor.matmul(sps[:], kt_sb[:, ts(kb, 128)],
                                 qt_sb[h][:, ds(512 * qc, 512)],
                                 start=True, stop=True)
                nc.scalar.activation(
                    out=et[:], in_=sps[:],
                    func=mybir.ActivationFunctionType.Exp,
                    scale=float(D) ** -0.5)
                return et, 0

            def emit_attn(qc, enq, pre, post):
                """enq: head -> [("loads", t) | ("wo", t) | ("wol", ...)]
                run at head start.  pre/post: head -> [proj thunks] at
                head start / head end."""
                nkb = 4 * qc + 4
                for h in range(G):
                    for act in enq.get(h, ()):
                        if act[0] == "loads":
                            wo_loads(act[1])
                        elif act[0] == "wo":
                            queue_wo(act[1])
                        else:
                            for kt in act[1]:
                                load_wo(kt)
                    for g in pre.get(h, ()):
                        g()
                    oT_ps = ps_pv.tile([128, 512], F32, tag="pv", name="pv")
                    pend = [emit_st(h, qc, k)
                            for k in range(min(LOOKAHEAD, nkb))]
                    # pairwise-tree accumulation of exp tiles (bf16)
                    ladder = []

                    def ladd(t_):
                        cur, lvl = t_, 0
                        while lvl < len(ladder) and ladder[lvl] is not None:
                            prev = ladder[lvl]
                            ladder[lvl] = None
                            ns = etwork.tile([128, 512], BF, tag="esum",
                                             name="esum", bufs=6)
                            nc.vector.tensor_add(ns[:], prev[:], cur[:])
                            cur = ns
                            lvl += 1
                        if lvl == len(ladder):
                            ladder.append(None)
                        ladder[lvl] = cur

                    for kb in range(nkb):
                        if kb + LOOKAHEAD < nkb:
                            pend.append(emit_st(h, qc, kb + LOOKAHEAD))
                        et, off = pend.pop(0)
                        nc.tensor.matmul(oT_ps[:, ds(off, 512 - off)],
                                         vtok_sb[:, kb, :],
                                         et[:, ds(off, 512 - off)],
                                         start=(kb == 0), stop=(kb == nkb - 1))
                        pop_fill(2)
                        ladd(et)
                    parts = [t_ for t_ in ladder if t_ is not None]
                    cur = parts[0]
                    for p_ in parts[1:]:
                        ns = etwork.tile([128, 512], BF, tag="esum",
                                         name="esum", bufs=6)
                        nc.vector.tensor_add(ns[:], cur[:], p_[:])
                        cur = ns
                    ar = stats.tile([128, 512], F32, tag="ar", name="ar")
                    nc.gpsimd.partition_all_reduce(ar[:], cur[:], 128,
                                                   bass_isa.ReduceOp.add)
                    rec = stats.tile([128, 512], F32, tag="rec", name="rec")
                    nc.vector.reciprocal_approx_fast(out=rec[:], in_=ar[:])
                    otst = work.tile([128, 512], BF, tag="otst", name="otst")
                    nc.vector.tensor_mul(otst[:], oT_ps[:], rec[:])
                    if qc == 3:
                        nc.gpsimd.dma_start(out=cin3[h][:], in_=otst[:])
                        nc.gpsimd.collective_compute(
                            "AllGather", mybir.AluOpType.bypass,
                            replica_groups=groups,
                            ins=[cin3[h].opt()], outs=[cout3[h].opt()])
                    else:
                        nc.gpsimd.dma_start(out=cin[qc][:, h, :], in_=otst[:])
                    pop_fill(4)
                    for g in post.get(h, ()):
                        g()
                if qc != 3:
                    nc.gpsimd.collective_compute(
                        "AllGather", mybir.AluOpType.bypass,
                        replica_groups=groups,
                        ins=[cin[qc].opt()], outs=[cout[qc].opt()])

            # ---------------- schedule
            pg = {c: proj_groups(c) for c in range(TCH)}
            pg[0]["k"]()
            pg[0]["q0"]()
            pg[0]["v"]()
            # chunk-0 q1..q3 + all of chunk 1 fold into quarter 0: each
            # head's q is emitted one head-start early so the rope chain
            # latency hides under the previous head's attention.  wo
            # weight loads land late in quarter 0.
            emit_attn(0, {2: [("wol", range(0, 8))],
                          3: [("wol", range(8, 16))]},
                      {0: [pg[0]["q1"]], 1: [pg[0]["q2"]],
                       2: [pg[0]["q3"]], 3: [pg[1]["v"]]},
                      {0: [pg[1]["k"]], 1: [pg[1]["q0"]],
                       2: [pg[1]["q1"]], 3: [pg[1]["q2"]]})
            emit_attn(1, {0: [("loads", 0)], 3: [("wo", 0)]},
                      {0: [pg[1]["q3"]], 2: [pg[2]["q1"]],
                       3: [pg[2]["v"]]},
                      {0: [pg[2]["k"]], 1: [pg[2]["q0"]],
                       2: [pg[2]["q2"]], 3: [pg[2]["q3"]]})
            emit_attn(2, {0: [("loads", 1)], 3: [("wo", 1)]},
                      {1: [pg[3]["q0"]], 2: [pg[3]["q2"]],
                       3: [pg[3]["v"]]},
                      {0: [pg[3]["k"]], 1: [pg[3]["q1"]],
                       2: [pg[3]["q3"]], 3: []})
            emit_attn(3, {0: [("loads", 2)], 3: [("wo", 2)]}, {}, {})
            pop_fill(len(pe_fill))

            # ---------------- tail: four Wo waves, one per gathered head
            pw = [None] * 4
            for h in range(G):
                ogs = []
                for r in range(4):
                    o = ogpool.tile([128, 512], BF, tag="og", name="og")
                    nc.sync.dma_start(out=o[:], in_=cout3[h][r, :, :])
                    ogs.append(o)
                for tb in range(4):
                    if h == 0:
                        pool = [ps_wo, ps_pv, ps_pv, ps_pj][tb]
                        tag = ["wo", "pv", "pv", "pj"][tb]
                        pw[tb] = pool.tile([128, 512], F32, tag=tag,
                                           name="pwt")
                    for r in range(4):
                        nc.tensor.matmul(pw[tb][:],
                                         ogs[r][:, ts(tb, 128)],
                                         wo_sb[r * G + h][:],
                                         start=(h == 0 and r == 0),
                                         stop=(h == G - 1 and r == 3))
                    if h == G - 1:
                        ost = work.tile([128, 512], F32, tag="ost",
                                        name="ost", bufs=3)
                        nc.vector.tensor_copy(out=ost[:], in_=pw[tb][:])
                        nc.sync.dma_start(
                            out=out[ds(512 * 3 + 128 * tb, 128), :],
                            in_=ost[:])

    nc.finalize()
    return nc


_NC_CACHE = {}


def _get_nc():
    if "nc" not in _NC_CACHE:
        _NC_CACHE["nc"] = build_nc()
    return _NC_CACHE["nc"]


def _rope_tables():
    inv = 1.0 / (THETA ** (np.arange(0, D, 2, dtype=np.float64) / D))  # [64]
    pos = np.arange(S, dtype=np.float64)
    fr = pos[:, None] * inv[None, :]                 # [S, 64]
    emb = np.concatenate([fr, fr], axis=1)           # [S, D]
    cos = np.cos(emb).T.astype(np.float32)           # [D, S]
    sin = np.sin(emb).T.astype(np.float32)
    sgn = np.where(np.arange(D) < D // 2, -1.0, 1.0).astype(np.float32)[:, None]
    sink = sin * sgn                                 # sign-folded sin
    # pre-shift by 64 partitions: u[p] = raw[p]*sink[(p+64)%128] then a
    # 64-partition rotation of u gives rotate_half(raw)*sink exactly
    sink_pre = np.roll(sink, 64, axis=0)
    return cos.copy(), sink_pre.copy()


def kernel(x, Wq, Wk, Wv, Wo):
    global LAST_EXEC_TIME_NS, LAST_RESULTS
    nc = _get_nc()
    ck, sp = _rope_tables()
    # causal-mask lhsT: (cmut.T @ iden)[p, j] = NEG iff j < p
    cmut_np = np.where(np.arange(128)[None, :] > np.arange(128)[:, None],
                       np.float32(NEG), np.float32(0.0)).astype(BF16)
    iden_np = np.eye(128, dtype=np.float32).astype(BF16)
    in_maps = []
    for c in range(N_CORES):
        b, kh = c // 4, c % 4
        in_maps.append({
            "xt": np.ascontiguousarray(x[b].T).astype(BF16),
            "wq": np.ascontiguousarray(Wq[:, kh * HD_CORE:(kh + 1) * HD_CORE]).astype(BF16),
            "wk": np.ascontiguousarray(Wk[:, kh * D:(kh + 1) * D]).astype(BF16),
            "wv": np.ascontiguousarray(Wv[:, kh * D:(kh + 1) * D]).astype(BF16),
            "wo": np.ascontiguousarray(Wo[:, kh * HD_CORE:(kh + 1) * HD_CORE]).astype(BF16),
            "cosk": ck, "sink": sp, "cmut": cmut_np, "iden": iden_np,
        })
    trace = os.environ.get("KERNEL_TRACE", "0") == "1" and _install_ntff_hook()
    if os.environ.get("KERNEL_WARMUP", "1") == "1":
        # Untraced warm-up execution: first-launch NEFF load/JIT skews the 8
        # cores by 10-100us, which lands in core 0's collective waits.
        run_bass_kernel_spmd(nc, in_maps, core_ids=list(range(N_CORES)),
                             trace=False)
    res = run_bass_kernel_spmd(nc, in_maps, core_ids=list(range(N_CORES)),
                               trace=trace)
    LAST_EXEC_TIME_NS = res.exec_time_ns
    LAST_RESULTS = res
    out = np.empty((B, S, DM), dtype=np.float32)
    for c in range(N_CORES):
        b, kh = c // 4, c % 4
        out[b, :, kh * HD_CORE:(kh + 1) * HD_CORE] = res.results[c]["out"]
    return out


# revision 7
# speedup vs baseline: 1.0306x; 1.0202x over previous
"""Distributed Trainium2 kernel for causal GQA attention with RoPE.

Model: B=2, S=2048, DM=2048, H=16 q-heads, HK=4 kv-heads, D=128.
Sharding over 8 NeuronCores: core c = (batch b=c//4, kv-head kh=c%4).
Each core computes its 4 q-heads / 1 kv-head of one batch end-to-end,
AllGathers attention outputs within its 4-core batch group, and applies
a column slice of Wo, producing out[b][:, kh*512:(kh+1)*512].

v3 notes:
- all inputs host-pre-tiled so every DMA is a single fully-contiguous
  block; rope tables are bf16 per-chunk tiles (chunk 0 first).
- the 1/sqrt(D) score scale is folded into Wk on the host, so every exp
  runs with scale=1 and shares the warm-up exp's scale/bias constant
  (a scale constant would otherwise be DMA'd behind the startup flood).
- PE warm-up matmuls at t=0 open the HAM clock gate during the
  DMA-bound startup; chunk-0 q1..q3 projections interleave into
  quarter-0 attention (one proj group per head-start/head-end slot).
- rope: DVE copies raw PSUM->SBUF bf16 (frees the proj PSUM bank fast),
  bf16 muls with bf16 tables, 64-partition shift via vector-queue DMA.
- causal mask folded into the score matmul (NEG upper-triangle lhsT @
  identity, accumulated before k.T@q) - no DVE mask add.
- PSUM banks: scores 3 (lookahead 2), proj 1, PV 2, den 1, Wo 1.
- tail: quarter 3 gathers per head right after each head's output; Wo
  accumulates in 4 per-token-block PSUM tiles across 4 waves so only
  the last head's 16 matmuls trail the final gather.
"""
import contextlib
import ctypes
import os
import sys
import types
from collections import deque

for _p in ("/opt/trn_rl_repo", "/root/.axon_site/_ro/trn_rl_repo"):
    if os.path.isdir(_p) and _p not in sys.path:
        sys.path.insert(0, _p)

import numpy as np
import ml_dtypes

import concourse.bass as bass
import concourse.mybir as mybir
import concourse.tile as tile
from concourse import bacc
from concourse.bass import ts, ds
from concourse.bass_utils import run_bass_kernel_spmd

BF16 = ml_dtypes.bfloat16
F32 = mybir.dt.float32
BF = mybir.dt.bfloat16

B, S, DM = 2, 2048, 2048
H, HK, D = 16, 4, 128
G = H // HK          # q heads per kv head (= heads per core)
THETA = 10000.0
N_CORES = 8
KT = DM // 128       # 16 K-tiles of the model dim
TOKB = S // 128      # 16 token blocks
TCH = S // 512       # 4 token chunks of 512
HD_CORE = G * D      # 512 output dims of q per core
NEG = -1.0e30
LOOKAHEAD = 2        # score tiles in flight ahead of PV

LAST_EXEC_TIME_NS = None
LAST_RESULTS = None


# ---------------------------------------------------------------- tracing
def _install_ntff_hook():
    """Make run_bass_kernel_spmd(trace=True) work in this container."""
    try:
        from antenv.axon_hooks import get_axon_ntff_profile_hook  # noqa: F401
        return True
    except ImportError:
        pass
    so_path = "/opt/axon/libaxon_pjrt.so"
    if not os.path.exists(so_path):
        return False
    lib = ctypes.CDLL(so_path)
    if not hasattr(lib, "axon_start_nrt_profile"):
        return False
    lib.axon_start_nrt_profile.argtypes = [ctypes.POINTER(ctypes.c_int64), ctypes.c_size_t]
    lib.axon_start_nrt_profile.restype = ctypes.c_int64
    lib.axon_stop_nrt_profile.argtypes = [ctypes.c_char_p]
    lib.axon_stop_nrt_profile.restype = ctypes.c_int64

    @contextlib.contextmanager
    def _hook(output_dir, device_ids):
        import jax
        jax.devices()
        if device_ids:
            ids = (ctypes.c_int64 * len(device_ids))(*device_ids)
            rc = lib.axon_start_nrt_profile(ids, len(device_ids))
        else:
            rc = lib.axon_start_nrt_profile(None, 0)
        if rc != 0:
            raise RuntimeError(f"axon_start_nrt_profile rc={rc}")
        try:
            yield
        finally:
            n = lib.axon_stop_nrt_profile(str(output_dir).encode())
            print(f"profile: {n} file(s) in {output_dir}", file=sys.stderr)

    mod = types.ModuleType("antenv.axon_hooks")
    holder = {"h": _hook}
    mod.set_axon_ntff_profile_hook = lambda h: holder.__setitem__("h", h)
    mod.get_axon_ntff_profile_hook = lambda: holder.get("h")
    sys.modules["antenv.axon_hooks"] = mod
    import antenv
    antenv.axon_hooks = mod
    import concourse.bass_utils as bu
    bu.upload_artifacts = lambda tmpdir: str(tmpdir)
    return True


# ---------------------------------------------------------------- graph
def build_nc():
    nc = bacc.Bacc("TRN2", target_bir_lowering=False, debug=False,
                   num_devices=N_CORES)

    xt = nc.dram_tensor("xt", [TCH, KT, 128, 512], BF,
                        kind="ExternalInput").ap()
    wq = nc.dram_tensor("wq", [G, KT, 128, 128], BF,
                        kind="ExternalInput").ap()
    wk = nc.dram_tensor("wk", [KT, 128, D], BF, kind="ExternalInput").ap()
    wv = nc.dram_tensor("wv", [KT, 128, D], BF, kind="ExternalInput").ap()
    wo = nc.dram_tensor("wo", [KT, 128, HD_CORE], BF,
                        kind="ExternalInput").ap()
    cosb = nc.dram_tensor("cosb", [TCH, D, 512], BF,
                          kind="ExternalInput").ap()
    sinb = nc.dram_tensor("sinb", [TCH, D, 512], BF,
                          kind="ExternalInput").ap()
    cmut = nc.dram_tensor("cmut", [128, 128], BF, kind="ExternalInput").ap()
    iden = nc.dram_tensor("iden", [128, 128], BF, kind="ExternalInput").ap()
    out = nc.dram_tensor("out", [S, HD_CORE], F32, kind="ExternalOutput").ap()

    groups = [[0, 1, 2, 3], [4, 5, 6, 7]]

    with tile.TileContext(nc) as tc:
        with tc.tile_pool(name="const", bufs=1) as cpool, \
             tc.tile_pool(name="wts", bufs=1) as wpool, \
             tc.tile_pool(name="acts", bufs=1) as apool, \
             tc.tile_pool(name="xin", bufs=64) as xpool, \
             tc.tile_pool(name="work", bufs=2) as work, \
             tc.tile_pool(name="etwork", bufs=7) as etwork, \
             tc.tile_pool(name="ogp", bufs=20) as ogpool, \
             tc.tile_pool(name="stats", bufs=2) as stats, \
             tc.tile_pool(name="bcp", bufs=2) as bcpool, \
             tc.tile_pool(name="pssc", bufs=3, space="PSUM") as ps_sc, \
             tc.tile_pool(name="pspj", bufs=1, space="PSUM") as ps_pj, \
             tc.tile_pool(name="pspv", bufs=2, space="PSUM") as ps_pv, \
             tc.tile_pool(name="psden", bufs=1, space="PSUM") as ps_den, \
             tc.tile_pool(name="pswo", bufs=1, space="PSUM") as ps_wo, \
             tc.tile_pool(name="dram", bufs=1, space="DRAM") as dpool:

            # ---------------- constants (host-built, tiny: land first)
            cmut_sb = cpool.tile([128, 128], BF, tag="cmut", name="cmut")
            nc.scalar.dma_start(out=cmut_sb[:], in_=cmut[:])
            iden_sb = cpool.tile([128, 128], BF, tag="iden", name="iden")
            nc.scalar.dma_start(out=iden_sb[:], in_=iden[:])

            # warm the ACT exp table so the first real exp is fast
            warm_act = cpool.tile([128, 1], F32, tag="warm_act",
                                  name="warm_act")
            nc.gpsimd.memset(warm_act[:], 0.0)
            nc.scalar.activation(out=warm_act[:], in_=warm_act[:],
                                 func=mybir.ActivationFunctionType.Exp)
            ones_sb = cpool.tile([128, 1], BF, tag="ones", name="ones")
            nc.gpsimd.memset(ones_sb[:], 1.0)

            # PE warm-up: ~3.5us of dependency-free matmuls at t=0 so the
            # HAM clock-gate opens to 8/8 during the DMA-bound startup
            warm_rhs = cpool.tile([128, 512], BF, tag="warm_rhs",
                                  name="warm_rhs")
            nc.gpsimd.memset(warm_rhs[:], 0.0)
            warm_ps = ps_wo.tile([128, 512], F32, tag="wo", name="warm_ps")
            for _ in range(8):
                nc.tensor.matmul(warm_ps[:], warm_rhs[:, 0:128],
                                 warm_rhs[:], start=True, stop=True)

            # ---------------- rope tables: per-chunk bf16, chunk 0 first
            cos_t = [None] * TCH
            sin_t = [None] * TCH

            def load_tbl(c):
                ct = cpool.tile([D, 512], BF, tag=f"cos{c}", name=f"cos{c}")
                nc.gpsimd.dma_start(out=ct[:], in_=cosb[c])
                st = cpool.tile([D, 512], BF, tag=f"sin{c}", name=f"sin{c}")
                nc.gpsimd.dma_start(out=st[:], in_=sinb[c])
                cos_t[c], sin_t[c] = ct, st

            load_tbl(0)
            # warm up the collective path early
            warm_in = dpool.tile([128, 4], BF, tag="warm_in", name="warm_in")
            warm_out = dpool.tile([4, 128, 4], BF, tag="warm_out",
                                  name="warm_out")
            nc.gpsimd.dma_start(out=warm_in[:], in_=cosb[0, :, 0:4])
            nc.gpsimd.collective_compute(
                "AllGather", mybir.AluOpType.bypass,
                replica_groups=groups,
                ins=[warm_in.opt()], outs=[warm_out.opt()])
            load_tbl(1)
            load_tbl(2)
            load_tbl(3)

            # ---------------- weights: scalar queue, critical first.
            # wo loads are deferred into quarter 1 ("wol" slots).
            wk_sb = [wpool.tile([128, D], BF, tag=f"wk{kt}",
                                name=f"wk{kt}") for kt in range(KT)]
            wq_sb = [[wpool.tile([128, 128], BF, tag=f"wq{h}_{kt}",
                                 name=f"wq{h}_{kt}") for kt in range(KT)]
                     for h in range(G)]
            wv_sb = [wpool.tile([128, D], BF, tag=f"wv{kt}",
                                name=f"wv{kt}") for kt in range(KT)]
            wo_sb = [wpool.tile([128, HD_CORE], BF, tag=f"wo{kt}",
                                name=f"wo{kt}") for kt in range(KT)]

            for kt in range(KT):
                nc.scalar.dma_start(out=wk_sb[kt][:], in_=wk[kt])
            for kt in range(KT):
                nc.scalar.dma_start(out=wq_sb[0][kt][:], in_=wq[0, kt])
            for kt in range(KT):
                nc.scalar.dma_start(out=wv_sb[kt][:], in_=wv[kt])
            for h in range(1, G):
                for kt in range(KT):
                    nc.scalar.dma_start(out=wq_sb[h][kt][:], in_=wq[h, kt])

            def load_xc(c):
                ts_ = [xpool.tile([128, 512], BF, tag="xc", name="xc")
                       for _ in range(KT)]
                for kt in range(KT):
                    nc.sync.dma_start(out=ts_[kt][:], in_=xt[c, kt])
                return ts_

            xc_state = [load_xc(c) for c in range(TCH)]

            def load_wo(kt):
                nc.gpsimd.dma_start(out=wo_sb[kt][:], in_=wo[kt])

            # ---------------- persistent activations
            qt_sb = [apool.tile([D, S], BF, tag=f"qt{h}", name=f"qt{h}")
                     for h in range(G)]
            kt_sb = apool.tile([D, S], BF, tag="kt", name="kt")
            vtok_sb = apool.tile([128, TOKB, D], BF, tag="vtok", name="vtok")

            # ---------------- projections + RoPE + direct token-major v
            def rope_store(raw_ps, c, dst_slice):
                # copy raw to SBUF bf16 (frees the proj PSUM bank), then
                # u = raw*sin_pre, 64-partition shift of u via DMA, and
                # dst = shifted + raw*cos.  sin table pre-shifted on host.
                raw = work.tile([128, 512], BF, tag="rawb", name="rawb")
                nc.vector.tensor_copy(out=raw[:], in_=raw_ps)
                u = work.tile([128, 512], BF, tag="u", name="u")
                nc.vector.tensor_mul(u[:], raw[:], sin_t[c][:])
                sh = work.tile([128, 512], BF, tag="sh", name="sh")
                nc.gpsimd.dma_start(out=sh[0:64, :], in_=u[64:128, :])
                nc.gpsimd.dma_start(out=sh[64:128, :], in_=u[0:64, :])
                t2 = work.tile([128, 512], BF, tag="t2", name="t2")
                nc.vector.tensor_mul(t2[:], raw[:], cos_t[c][:])
                nc.vector.tensor_add(dst_slice, sh[:], t2[:])

            def proj_groups(c):
                """Chunk c's projection as 6 thunks (k, q0..q3, v)."""
                def g_k():
                    xc = xc_state[c]
                    ps = ps_pj.tile([128, 512], F32, tag="pj", name="pj")
                    for kt in range(KT):
                        nc.tensor.matmul(ps[:], wk_sb[kt][:], xc[kt][:],
                                         start=(kt == 0), stop=(kt == KT - 1))
                    rope_store(ps[:], c, kt_sb[:, ds(512 * c, 512)])

                def mk_q(h):
                    def g_q():
                        xc = xc_state[c]
                        ps = ps_pj.tile([128, 512], F32, tag="pj", name="pj")
                        for kt in range(KT):
                            nc.tensor.matmul(ps[:], wq_sb[h][kt][:],
                                             xc[kt][:],
                                             start=(kt == 0),
                                             stop=(kt == KT - 1))
                        rope_store(ps[:], c, qt_sb[h][:, ds(512 * c, 512)])
                    return g_q

                def g_v():
                    xc = xc_state[c]
                    ps = ps_pj.tile([128, 512], F32, tag="pj", name="pj")
                    for tb in range(4):
                        for kt in range(KT):
                            nc.tensor.matmul(ps[:, ts(tb, 128)],
                                             xc[kt][:, ts(tb, 128)],
                                             wv_sb[kt][:],
                                             start=(kt == 0),
                                             stop=(kt == KT - 1))
                    nc.vector.tensor_copy(out=vtok_sb[:, ds(4 * c, 4), :],
                                          in_=ps[:])

                return {"k": g_k, "q0": mk_q(0), "q1": mk_q(1),
                        "q2": mk_q(2), "q3": mk_q(3), "v": g_v}

            # ---------------- collective buffers
            cin = [dpool.tile([D, G, 512], BF, tag=f"cin{t}", name=f"cin{t}")
                   for t in range(3)]
            cout = [dpool.tile([4, D, G, 512], BF, tag=f"cout{t}",
                               name=f"cout{t}") for t in range(3)]
            cin3 = [dpool.tile([D, 512], BF, tag=f"cin3{h}", name=f"cin3{h}")
                    for h in range(G)]
            cout3 = [dpool.tile([4, D, 512], BF, tag=f"cout3{h}",
                                name=f"cout3{h}") for h in range(G)]

            # ---------------- Wo: og loads + filler-granular matmuls
            og = {}

            def wo_loads(t):
                ogs = []
                for kt in range(KT):
                    r, h = divmod(kt, G)
                    o = ogpool.tile([128, 512], BF, tag="og", name="og")
                    nc.sync.dma_start(out=o[:], in_=cout[t][r, :, h, :])
                    ogs.append(o)
                og[t] = ogs

            pe_fill = deque()

            def queue_wo(t):
                """Enqueue quarter t's Wo work as single-matmul closures."""
                for tb in range(4):
                    state = {}

                    def mk(tb, pos, state):
                        def f():
                            if pos == 0:
                                state["pw"] = ps_wo.tile([128, 512], F32,
                                                         tag="wo", name="wo")
                            nc.tensor.matmul(state["pw"][:],
                                             og[t][pos][:, ts(tb, 128)],
                                             wo_sb[pos][:],
                                             start=(pos == 0),
                                             stop=(pos == KT - 1))
                            if pos == KT - 1:
                                ost = work.tile([128, 512], F32, tag="ost",
                                                name="ost", bufs=3)
                                nc.vector.tensor_copy(out=ost[:],
                                                      in_=state["pw"][:])
                                nc.sync.dma_start(
                                    out=out[ds(512 * t + 128 * tb, 128), :],
                                    in_=ost[:])
                        return f

                    for pos in range(KT):
                        pe_fill.append(mk(tb, pos, state))

            def pop_fill(n):
                for _ in range(n):
                    if not pe_fill:
                        return
                    pe_fill.popleft()()

            # ---------------- attention
            def emit_st(h, qc, kb):
                """score block, transposed: [k 128, q<=512] -> exp -> et.
                Causal mask accumulated via matmul (cmut.T @ iden); score
                scale is pre-folded into Wk on the host."""
                band = kb - 4 * qc
                et = etwork.tile([128, 512], BF, tag="et", name="et")
                sps = ps_sc.tile([128, 512], F32, tag="mm", name="mm")
                if band >= 0:
                    off = 128 * band
                    w = 512 - off
                    nc.tensor.matmul(sps[:, 0:128], cmut_sb[:], iden_sb[:],
                                     start=True, stop=False)
                    nc.tensor.matmul(sps[:, :w], kt_sb[:, ts(kb, 128)],
                                     qt_sb[h][:, ds(512 * qc + off, w)],
                                     start=False, stop=True)
                    if off:
                        nc.vector.memset(et[:, :off], 0.0)
                    nc.scalar.activation(
                        out=et[:, ds(off, w)], in_=sps[:, :w],
                        func=mybir.ActivationFunctionType.Exp)
                    return et, off
                nc.tensor.matmul(sps[:], kt_sb[:, ts(kb, 128)],
                                 qt_sb[h][:, ds(512 * qc, 512)],
                                 start=True, stop=True)
                nc.scalar.activation(
                    out=et[:], in_=sps[:],
                    func=mybir.ActivationFunctionType.Exp)
                return et, 0

            def emit_attn(qc, enq, pre, post):
                """enq: head -> [("loads", t) | ("wo", t) | ("wol", ...)]
                run at head start.  pre/post: head -> [proj thunks] at
                head start / head end."""
                nkb = 4 * qc + 4
                for h in range(G):
                    for act in enq.get(h, ()):
                        if act[0] == "loads":
                            wo_loads(act[1])
                        elif act[0] == "wo":
                            queue_wo(act[1])
                        else:
                            for kt in act[1]:
                                load_wo(kt)
                    for g in pre.get(h, ()):
                        g()
                    oT_ps = ps_pv.tile([128, 512], F32, tag="pv", name="pv")
                    den_ps = ps_den.tile([1, 512], F32, tag="den", name="den")
                    pend = [emit_st(h, qc, k)
                            for k in range(min(LOOKAHEAD, nkb))]
                    ngrp = (nkb + 3) // 4
                    esum = None
                    for kb in range(nkb):
                        if kb + LOOKAHEAD < nkb:
                            pend.append(emit_st(h, qc, kb + LOOKAHEAD))
                        et, off = pend.pop(0)
                        nc.tensor.matmul(oT_ps[:, ds(off, 512 - off)],
                                         vtok_sb[:, kb, :],
                                         et[:, ds(off, 512 - off)],
                                         start=(kb == 0), stop=(kb == nkb - 1))
                        pop_fill(2)
                        # denominator: sum groups of 4 et tiles on DVE,
                        # then one ones-matmul per group
                        gi, gj = divmod(kb, 4)
                        last_in_grp = (gj == 3 or kb == nkb - 1)
                        if gj == 0:
                            esum = et
                        else:
                            nsum = etwork.tile([128, 512], BF, tag="esum",
                                               name="esum", bufs=3)
                            nc.vector.tensor_add(nsum[:], esum[:], et[:])
                            esum = nsum
                        if last_in_grp:
                            nc.tensor.matmul(den_ps[:], ones_sb[:, 0:1],
                                             esum[:],
                                             start=(gi == 0),
                                             stop=(gi == ngrp - 1))
                    rec = stats.tile([1, 512], F32, tag="recq", name="recq")
                    nc.vector.reciprocal_approx_fast(out=rec[:],
                                                     in_=den_ps[:])
                    bcast = bcpool.tile([128, 512], F32, tag="bcast",
                                        name="bcast")
                    nc.gpsimd.partition_broadcast(bcast[:], rec[:])
                    otst = work.tile([128, 512], BF, tag="otst", name="otst")
                    nc.vector.tensor_mul(otst[:], oT_ps[:], bcast[:])
                    if qc == 3:
                        nc.gpsimd.dma_start(out=cin3[h][:], in_=otst[:])
                        nc.gpsimd.collective_compute(
                            "AllGather", mybir.AluOpType.bypass,
                            replica_groups=groups,
                            ins=[cin3[h].opt()], outs=[cout3[h].opt()])
                    else:
                        nc.gpsimd.dma_start(out=cin[qc][:, h, :], in_=otst[:])
                    pop_fill(4)
                    for g in post.get(h, ()):
                        g()
                if qc != 3:
                    nc.gpsimd.collective_compute(
                        "AllGather", mybir.AluOpType.bypass,
                        replica_groups=groups,
                        ins=[cin[qc].opt()], outs=[cout[qc].opt()])

            # ---------------- schedule
            pg = {c: proj_groups(c) for c in range(TCH)}
            pg[0]["k"]()
            pg[0]["q0"]()
            pg[0]["v"]()
            # chunk-0 q1..q3 + all of chunk 1 fold into quarter 0: each
            # head's q is emitted one head early so the rope latency
            # hides under the previous head's attention.
            emit_attn(0, {},
                      {0: [pg[0]["q1"]], 1: [pg[0]["q2"]],
                       2: [pg[0]["q3"]], 3: [pg[1]["v"]]},
                      {0: [pg[1]["k"]], 1: [pg[1]["q0"]],
                       2: [pg[1]["q1"]], 3: [pg[1]["q2"]]})
            emit_attn(1, {0: [("loads", 0), ("wol", range(0, 8))],
                          1: [("wol", range(8, 16))],
                          3: [("wo", 0)]},
                      {0: [pg[1]["q3"]], 2: [pg[2]["q1"]],
                       3: [pg[2]["v"]]},
                      {0: [pg[2]["k"]], 1: [pg[2]["q0"]],
                       2: [pg[2]["q2"]], 3: [pg[2]["q3"]]})
            emit_attn(2, {0: [("loads", 1)], 3: [("wo", 1)]},
                      {1: [pg[3]["q0"]], 2: [pg[3]["q2"]],
                       3: [pg[3]["v"]]},
                      {0: [pg[3]["k"]], 1: [pg[3]["q1"]],
                       2: [pg[3]["q3"]], 3: []})
            emit_attn(3, {0: [("loads", 2)], 3: [("wo", 2)]}, {}, {})
            pop_fill(len(pe_fill))

            # ---------------- tail: four Wo waves, one per gathered head
            pw = [None] * 4
            for h in range(G):
                ogs = []
                for r in range(4):
                    o = ogpool.tile([128, 512], BF, tag="og", name="og")
                    nc.sync.dma_start(out=o[:], in_=cout3[h][r, :, :])
                    ogs.append(o)
                for tb in range(4):
                    if h == 0:
                        pool = [ps_wo, ps_pv, ps_pv, ps_sc][tb]
                        tag = ["wo", "pv", "pv", "mm"][tb]
                        pw[tb] = pool.tile([128, 512], F32, tag=tag,
                                           name="pwt")
                    for r in range(4):
                        nc.tensor.matmul(pw[tb][:],
                                         ogs[r][:, ts(tb, 128)],
                                         wo_sb[r * G + h][:],
                                         start=(h == 0 and r == 0),
                                         stop=(h == G - 1 and r == 3))
                    if h == G - 1:
                        ost = work.tile([128, 512], F32, tag="ost",
                                        name="ost", bufs=3)
                        nc.vector.tensor_copy(out=ost[:], in_=pw[tb][:])
                        nc.sync.dma_start(
                            out=out[ds(512 * 3 + 128 * tb, 128), :],
                            in_=ost[:])

    nc.finalize()
    return nc


_NC_CACHE = {}


def _get_nc():
    if "nc" not in _NC_CACHE:
        _NC_CACHE["nc"] = build_nc()
    return _NC_CACHE["nc"]


def _rope_tables():
    inv = 1.0 / (THETA ** (np.arange(0, D, 2, dtype=np.float64) / D))  # [64]
    pos = np.arange(S, dtype=np.float64)
    fr = pos[:, None] * inv[None, :]                 # [S, 64]
    emb = np.concatenate([fr, fr], axis=1)           # [S, D]
    cos = np.cos(emb).T.astype(np.float32)           # [D, S]
    sin = np.sin(emb).T.astype(np.float32)
    sgn = np.where(np.arange(D) < D // 2, -1.0, 1.0).astype(np.float32)[:, None]
    sink = sin * sgn                                 # sign-folded sin
    # pre-shift by 64 partitions: u[p] = raw[p]*sink[(p+64)%128] then a
    # 64-partition rotation of u gives rotate_half(raw)*sink exactly
    sink_pre = np.roll(sink, 64, axis=0)
    # per-chunk bf16 tiles [TCH, D, 512]
    cos_t = np.ascontiguousarray(
        cos.reshape(D, TCH, 512).transpose(1, 0, 2)).astype(BF16)
    sin_t = np.ascontiguousarray(
        sink_pre.reshape(D, TCH, 512).transpose(1, 0, 2)).astype(BF16)
    return cos_t, sin_t


def kernel(x, Wq, Wk, Wv, Wo):
    global LAST_EXEC_TIME_NS, LAST_RESULTS
    nc = _get_nc()
    ct, st = _rope_tables()
    cmut_np = np.where(np.arange(128)[None, :] > np.arange(128)[:, None],
                       np.float32(NEG), np.float32(0.0)).astype(BF16)
    iden_np = np.eye(128, dtype=np.float32).astype(BF16)
    scale = np.float32(D ** -0.5)
    in_maps = []
    for c in range(N_CORES):
        b, kh = c // 4, c % 4
        xT = np.ascontiguousarray(x[b].T).astype(BF16)       # [DM, S]
        xtile = np.ascontiguousarray(
            xT.reshape(KT, 128, TCH, 512).transpose(2, 0, 1, 3))
        wq_s = Wq[:, kh * HD_CORE:(kh + 1) * HD_CORE]        # [DM, 512]
        wq_t = np.ascontiguousarray(
            wq_s.reshape(KT, 128, G, 128).transpose(2, 0, 1, 3)).astype(BF16)
        wk_t = np.ascontiguousarray(
            (Wk[:, kh * D:(kh + 1) * D] * scale).reshape(KT, 128, D)
        ).astype(BF16)
        wv_t = np.ascontiguousarray(
            Wv[:, kh * D:(kh + 1) * D].reshape(KT, 128, D)).astype(BF16)
        wo_t = np.ascontiguousarray(
            Wo[:, kh * HD_CORE:(kh + 1) * HD_CORE].reshape(KT, 128, HD_CORE)
        ).astype(BF16)
        in_maps.append({
            "xt": xtile, "wq": wq_t, "wk": wk_t, "wv": wv_t, "wo": wo_t,
            "cosb": ct, "sinb": st, "cmut": cmut_np, "iden": iden_np,
        })
    trace = os.environ.get("KERNEL_TRACE", "0") == "1" and _install_ntff_hook()
    if os.environ.get("KERNEL_WARMUP", "1") == "1":
        # Untraced warm-up execution: first-launch NEFF load/JIT skews the 8
        # cores by 10-100us, which lands in core 0's collective waits.
        run_bass_kernel_spmd(nc, in_maps, core_ids=list(range(N_CORES)),
                             trace=False)
    res = run_bass_kernel_spmd(nc, in_maps, core_ids=list(range(N_CORES)),
                               trace=trace)
    LAST_EXEC_TIME_NS = res.exec_time_ns
    LAST_RESULTS = res
    out = np.empty((B, S, DM), dtype=np.float32)
    for c in range(N_CORES):
        b, kh = c // 4, c % 4
        out[b, :, kh * HD_CORE:(kh + 1) * HD_CORE] = res.results[c]["out"]
    return out
